# revision 1
# baseline (speedup 1.0000x reference)
"""GCL (GNN message-passing) Trainium2 Bass kernel on 8 NeuronCores.

Sharding: edges sorted by destination on host and sharded by destination-node
range (1250 nodes/core) -> each core owns the full segment-sum for its nodes,
no collectives. Node features and weights replicated.

Per core, per 128-destination-node window, edges are processed in 512-edge
macro tiles:
  e1T[D,e] = A_hi/A_lo @ S_T + I @ Bcol          (PSUM accumulate)
  where A = h@we1_top + be1 (double-bf16, resident SBUF),
        B = h@we1_bot (single-bf16 HBM table, per-edge dma_gather on col),
        S_T[n,e] = one-hot(row_local[e]==n) built via K=1 broadcast matmul
                   + DVE is_equal against a partition-iota.
  e2[e,D] = silu(e1) @ we2 + be2                 (be2 via K=1 ones x be2)
  aggT[D,n] += e2^T-scatter via lhsT=e2, rhs=S   (PSUM accumulate per window)
Node MLP + residual per 128-node tile, fp32.
"""
import sys
sys.path.insert(0, '/opt/trn_rl_repo')
import numpy as np
import ml_dtypes

N_NODES = 10000
N_EDGES = 640000
D = 128
NORM = 100.0
NCORES = 8
NPC = N_NODES // NCORES          # 1250 destination nodes per core
NWIN = 10                        # 128-node windows per core
CALL = 1024                      # edges per dma_gather call (= 2 macros)
MACRO = 512
PAD_ROWLOCAL = 200.0

BF16 = ml_dtypes.bfloat16
_prog_cache = {}


def _wrap_idx16(idx):
    """[n] int -> [128, n/16] int16 wrapped (pos i -> partition i%16, col
    i//16) and replicated into all eight 16-partition groups."""
    n = idx.shape[0]
    block = idx.astype(np.int16).reshape(n // 16, 16).T
    return np.tile(block, (8, 1))


QPAT = [0, 1, 0, 2, 0, 1, 0, 3]   # queue shares ~ 1/cost: cost_q ~ (q+1)


def _build_program(cw_per_window, no_gather=False, no_compute=False,
                   bufs_g=6, bufs_w=3):
    import concourse.bacc as bacc
    import concourse.mybir as mybir
    from concourse import tile

    dt = mybir.dt
    AF = mybir.ActivationFunctionType
    ALU = mybir.AluOpType

    nm_w = [2 * c for c in cw_per_window]
    NM = sum(nm_w)
    NCALLS = sum(cw_per_window)
    RBLK = (NM + 127) // 128

    nc = bacc.Bacc("TRN2", target_bir_lowering=False, debug=False,
                   num_devices=NCORES, num_swdge_queues=4)

    f32, bf16, i16 = dt.float32, dt.bfloat16, dt.int16
    din = lambda n, s, d=f32: nc.dram_tensor(n, s, d, kind="ExternalInput")
    hT = din("hT", [128, 10240])
    hT_slice = din("hT_slice", [128, NWIN * 128])
    h_slice = din("h_slice", [NWIN, 128, 128])
    we1_top = din("we1_top", [128, 128])
    we1_bot = din("we1_bot", [128, 128])
    be1_row = din("be1_row", [1, 128])
    we2_d = din("we2", [128, 128])
    we2b_d = din("we2_bf", [128, 128], bf16)
    be2rep4 = din("be2rep4", [1, 512])
    wn1_lo = din("wn1_lo", [128, 128])
    wn1_hi = din("wn1_hi", [128, 128])
    bn1_col = din("bn1_col", [128, 1])
    wn2_d = din("wn2", [128, 128])
    bn2_row = din("bn2_row", [1, 128])
    ones_row = din("ones_row", [1, 128])
    iota_col_d = din("iota_col", [128, 512])
    iota_part_d = din("iota_part", [128, 1])
    ident_bf_d = din("ident_bf", [128, 128], bf16)
    ident_f_d = din("ident_f", [128, 128])
    colidx_d = din("colidx", [128, 64 * NCALLS], i16)
    rowloc_c_d = din("rowloc_c", [128, 4 * NM])
    rowloc_r_d = din("rowloc_r", [NM, 512])
    out_d = nc.dram_tensor("out", [NWIN, 128, 128], f32, kind="ExternalOutput")

    NB = 80                                   # B table: 80*128 = 10240 rows
    B_hbm = nc.dram_tensor("B_scratch", [NB * 128, 128], f32)

    with tile.TileContext(nc) as tc:
        with (
            tc.tile_pool(name="persist", bufs=1) as pp,
            tc.tile_pool(name="work", bufs=bufs_w) as wp,
            tc.tile_pool(name="gout", bufs=bufs_g) as gp,
            tc.tile_pool(name="ps", bufs=2, space="PSUM") as psp,
        ):
            def load(t_dram, shape, dtype=f32):
                t = pp.tile(shape, dtype, tag=t_dram.name)
                nc.sync.dma_start(t[:], t_dram.ap())
                return t

            hT_t = load(hT, [128, 10240])
            hTs_t = load(hT_slice, [128, NWIN * 128])
            colidx_t = load(colidx_d, [128, 64 * NCALLS], i16)
            rowloc_c = load(rowloc_c_d, [128, 4 * NM])
            w1t = load(we1_top, [128, 128])
            w1b = load(we1_bot, [128, 128])
            be1r = load(be1_row, [1, 128])
            w2 = load(we2_d, [128, 128])
            w2b = load(we2b_d, [128, 128], bf16)
            be2r = load(be2rep4, [1, 512])
            wn1l = load(wn1_lo, [128, 128])
            wn1h = load(wn1_hi, [128, 128])
            bn1c = load(bn1_col, [128, 1])
            wn2t = load(wn2_d, [128, 128])
            bn2r = load(bn2_row, [1, 128])
            onesr = load(ones_row, [1, 128])
            iota_col = load(iota_col_d, [128, 512])
            iota_part = load(iota_part_d, [128, 1])
            ident_bf = load(ident_bf_d, [128, 128], bf16)
            ident_f = load(ident_f_d, [128, 128])
            hsl_t = pp.tile([128, NWIN, 128], f32, tag="h_slice")
            nc.sync.dma_start(hsl_t[:], h_slice.ap().rearrange("w p d -> p w d"))

            # ---- B table: h @ we1_bot -> bf16 rows in HBM ----
            bview = B_hbm.ap().rearrange("(g t p) d -> g p t d", g=10, t=8, p=128)
            for g in range(10):
                stage = wp.tile([128, 8, 128], f32, tag="bstage")
                for ts in range(8):
                    t = g * 8 + ts
                    bp = psp.tile([128, 128], f32, tag="e1")
                    nc.tensor.matmul(bp[:], hT_t[:, t * 128:(t + 1) * 128], w1b[:],
                                     start=True, stop=True)
                    nc.scalar.activation(stage[:, ts, :], bp[:], AF.Copy)
                nc.sync.dma_start(bview[g], stage[:])

            # ---- A table: h @ we1_top + be1, double-bf16, SBUF resident ----
            a_hi = pp.tile([128, NWIN, 128], bf16, tag="a_hi")
            a_lo = pp.tile([128, NWIN, 128], bf16, tag="a_lo")
            for w in range(NWIN):
                ap_ = psp.tile([128, 128], f32, tag="e1")
                nc.tensor.matmul(ap_[:], onesr[:], be1r[:], start=True, stop=False)
                nc.tensor.matmul(ap_[:], hTs_t[:, w * 128:(w + 1) * 128], w1t[:],
                                 start=False, stop=True)
                nc.scalar.activation(a_hi[:, w, :], ap_[:], AF.Copy)
                nc.vector.tensor_tensor(
                    a_lo[:, w, :], ap_[:], a_hi[:, w, :], ALU.subtract)

            # ---- edge phase ----
            agg_sb = None
            if not no_compute:
                agg_sb = pp.tile([128, NWIN, 128], f32, tag="aggsb")

            # flat macro list: (window, mw-in-window, nmw)
            macros = [(w, mw, nm_w[w]) for w in range(NWIN) for mw in range(nm_w[w])]
            NMtot = len(macros)
            NCH = (NMtot + 3) // 4            # rb chunks of 4 macros
            gts = {}
            rbs = {}
            agg_tiles = {}
            stash = {}
            PREF = 4

            def issue_gather(cc):
                if cc >= NCALLS:
                    return
                gt = gp.tile([128, CALL // 128, 128], f32, tag="g")
                if not no_gather:
                    nc.gpsimd.dma_gather(
                        gt[:], B_hbm.ap(), colidx_t[:, cc * 64:(cc + 1) * 64],
                        num_idxs=CALL, num_idxs_reg=CALL, elem_size=128,
                        transpose=False, single_packet=False,
                        queue_num=QPAT[cc % len(QPAT)],
                    )
                else:
                    nc.vector.tensor_copy(gt[:, 0, 0:8], ident_f[:, 0:8])
                gts[cc] = gt

            def issue_rb(j):
                if j >= NCH or no_compute:
                    return
                n4 = min(4, NMtot - 4 * j)
                rb = wp.tile([128, 4, 512], f32, tag="rb")
                src = rowloc_r_d.ap()[4 * j:4 * j + n4, :].rearrange(
                    "(o a) b -> o (a b)", o=1).broadcast_to((128, n4 * 512))
                nc.sync.dma_start(rb[:, 0:n4, :], src)
                rbs[j] = rb

            for p in range(PREF):
                issue_gather(p)
            issue_rb(0)

            def front(i):
                w, mw, nmw = macros[i]
                if i % 2 == 0:
                    issue_gather(i // 2 + PREF)
                if i % 4 == 0 and i > 0:
                    issue_rb(i // 4)
                gt = gts[i // 2]
                if no_compute:
                    if i % 2 == 0:
                        sink = wp.tile([128, 8], f32, tag="sink")
                        nc.vector.tensor_copy(sink[:], gt[:, 0, 0:8])
                    return
                if i % 4 == 0:
                    issue_rb(i // 4 + 1)
                rb = rbs[i // 4]
                st = wp.tile([128, 512], bf16, tag="st")
                nc.vector.tensor_scalar(
                    st[:], rb[:, i % 4, :], iota_part[:, 0:1], None, ALU.is_equal)
                e1p = psp.tile([128, 512], f32, tag="e1")
                nc.tensor.matmul(e1p[:], a_hi[:, w, :], st[:],
                                 start=True, stop=False, skip_group_check=True)
                nc.tensor.matmul(e1p[:], a_lo[:, w, :], st[:],
                                 start=False, stop=False, skip_group_check=True)
                for t in range(4):
                    # transpose-inject: e1p[:, t-slice] += gt[:, j, :]^T
                    nc.tensor.matmul(
                        e1p[:, t * 128:(t + 1) * 128],
                        gt[:, (i % 2) * 4 + t, :], ident_f[:],
                        start=False, stop=True, skip_group_check=True)
                e1s = wp.tile([128, 512], bf16, tag="e1s")
                nc.scalar.activation(e1s[:], e1p[:], AF.Silu)
                stash[i] = e1s

            def back(i):
                if no_compute:
                    return
                w, mw, nmw = macros[i]
                e1s = stash.pop(i)
                if mw == 0:
                    agg_new = psp.tile([128, 128], f32, tag="agg")
                    agg_tiles[w] = agg_new
                agg_ps = agg_tiles[w]
                s4 = wp.tile([128, 512], bf16, tag="s4")
                for t in range(4):
                    nc.vector.tensor_scalar(
                        s4[:, t * 128:(t + 1) * 128],
                        iota_col[:, t * 128:(t + 1) * 128],
                        rowloc_c[:, 4 * i + t:4 * i + t + 1],
                        None, ALU.is_equal)
                e2p = psp.tile([128, 512], f32, tag="e2")
                nc.tensor.matmul(e2p[:], onesr[:], be2r[:],
                                 start=True, stop=False, skip_group_check=True)
                for t in range(4):
                    nc.tensor.matmul(
                        e2p[:, t * 128:(t + 1) * 128],
                        e1s[:, t * 128:(t + 1) * 128], w2b[:],
                        start=False, stop=True, skip_group_check=True)
                e2s = wp.tile([128, 512], bf16, tag="e2s")
                nc.scalar.activation(e2s[:], e2p[:], AF.Silu)
                for t in range(4):
                    nc.tensor.matmul(
                        agg_ps[:],
                        e2s[:, t * 128:(t + 1) * 128],
                        s4[:, t * 128:(t + 1) * 128],
                        start=(mw == 0 and t == 0),
                        stop=(mw == nmw - 1 and t == 3),
                        skip_group_check=True)
                if mw == nmw - 1:
                    nc.scalar.activation(agg_sb[:, w, :], agg_ps[:], AF.Copy,
                                         scale=1.0 / NORM)

            for i in range(NMtot + 1):
                if i < NMtot:
                    front(i)
                if i >= 1:
                    back(i - 1)

            # ---- node phase ----
            if no_compute:
                for w in range(NWIN):
                    nc.sync.dma_start(out_d.ap()[w], hsl_t[:, w, :])
            for w in range(NWIN) if not no_compute else []:
                hp = psp.tile([128, 128], f32, tag="e1")
                nc.tensor.matmul(hp[:], wn1l[:], hTs_t[:, w * 128:(w + 1) * 128],
                                 start=True, stop=False)
                nc.tensor.matmul(hp[:], wn1h[:], agg_sb[:, w, :],
                                 start=False, stop=True)
                hs = wp.tile([128, 128], f32, tag="hs")
                nc.scalar.activation(hs[:], hp[:], AF.Silu, bias=bn1c[:, 0:1])
                op = psp.tile([128, 128], f32, tag="e2")
                nc.tensor.matmul(op[:], onesr[:], bn2r[:], start=True, stop=False)
                nc.tensor.matmul(op[:], hs[:], wn2t[:], start=False, stop=True)
                ot = wp.tile([128, 128], f32, tag="ot")
                nc.vector.tensor_tensor(ot[:], op[:], hsl_t[:, w, :], ALU.add)
                nc.sync.dma_start(out_d.ap()[w], ot[:])

    nc.compile()
    return nc


def _prep_inputs(h, edge_index, we1, be1, we2, be2, wn1, bn1, wn2, bn2):
    """Host-side shard/sort/pad. Returns (cw_per_window, per-core in_maps)."""
    h = np.asarray(h, np.float32)
    row = np.asarray(edge_index[0], np.int64).astype(np.int32)
    col = np.asarray(edge_index[1], np.int64).astype(np.int32)

    # per (core, window) edge lists
    core = row // NPC
    rl_g = row - core * NPC
    win = rl_g // 128
    rl = (rl_g % 128).astype(np.float32)

    counts = np.zeros((NCORES, NWIN), np.int64)
    per = [[None] * NWIN for _ in range(NCORES)]
    for cid in range(NCORES):
        msk = core == cid
        w_c, rl_c, col_c = win[msk], rl[msk], col[msk]
        for w in range(NWIN):
            wm = w_c == w
            per[cid][w] = (col_c[wm], rl_c[wm])
            counts[cid, w] = wm.sum()
    cw_per_window = tuple(int(-(-counts[:, w].max() // CALL)) for w in range(NWIN))

    nm_w = [2 * c for c in cw_per_window]
    NM = sum(nm_w)
    NCALLS = sum(cw_per_window)
    RBLK = (NM + 127) // 128

    hT_pad = np.zeros((128, 10240), np.float32)
    hT_pad[:, :N_NODES] = h.T
    iota_col = np.tile(np.arange(128, dtype=np.float32), 4)[None, :].repeat(128, 0)
    iota_part = np.arange(128, dtype=np.float32)[:, None].copy()
    ident_bf = np.eye(128, dtype=np.float32).astype(BF16)
    shared = {
        "hT": hT_pad,
        "we1_top": np.asarray(we1[:128], np.float32),
        "we1_bot": np.asarray(we1[128:], np.float32),
        "be1_row": np.asarray(be1, np.float32)[None, :],
        "we2": np.asarray(we2, np.float32),
        "be2rep4": np.tile(np.asarray(be2, np.float32), 4)[None, :],
        "wn1_lo": np.asarray(wn1[:128], np.float32),
        "wn1_hi": np.asarray(wn1[128:], np.float32),
        "bn1_col": np.asarray(bn1, np.float32)[:, None].copy(),
        "wn2": np.asarray(wn2, np.float32),
        "bn2_row": np.asarray(bn2, np.float32)[None, :],
        "ones_row": np.ones((1, 128), np.float32),
        "iota_col": iota_col.copy(),
        "iota_part": iota_part,
        "ident_bf": ident_bf,
        "ident_f": np.eye(128, dtype=np.float32),
        "we2_bf": np.asarray(we2, np.float32).astype(BF16),
    }

    in_maps = []
    for cid in range(NCORES):
        colidx = np.zeros((128, 64 * NCALLS), np.int16)
        rowloc_c = np.zeros((128, 4 * NM), np.float32)
        rowloc_r = np.zeros((NM, 512), np.float32)
        ci = 0
        mi = 0
        for w in range(NWIN):
            ccol, crl = per[cid][w]
            n_slots = cw_per_window[w] * CALL
            col_pad = np.zeros(n_slots, np.int32)
            rl_pad = np.full(n_slots, PAD_ROWLOCAL, np.float32)
            col_pad[:len(ccol)] = ccol
            rl_pad[:len(crl)] = crl
            for cc in range(cw_per_window[w]):
                colidx[:, ci * 64:ci * 64 + 64] = _wrap_idx16(
                    col_pad[cc * CALL:(cc + 1) * CALL])
                ci += 1
            for mm in range(2 * cw_per_window[w]):
                seg = rl_pad[mm * MACRO:(mm + 1) * MACRO]
                rowloc_c[:, 4 * mi:4 * mi + 4] = seg.reshape(4, 128).T
                rowloc_r[mi] = seg
                mi += 1
        base = cid * NPC
        hT_slice = hT_pad[:, base:base + NWIN * 128].copy()
        h_slice = np.zeros((NWIN, 128, 128), np.float32)
        hi = min(N_NODES, base + NWIN * 128)
        h_slice.reshape(NWIN * 128, 128)[:hi - base] = h[base:hi]
        in_maps.append({**shared, "hT_slice": hT_slice, "h_slice": h_slice,
                        "colidx": colidx, "rowloc_c": rowloc_c,
                        "rowloc_r": rowloc_r})
    return cw_per_window, in_maps


def kernel(**inputs):
    from concourse.bass_utils import run_bass_kernel_spmd

    cw, in_maps = _prep_inputs(**inputs)
    if cw not in _prog_cache:
        _prog_cache[cw] = _build_program(cw)
    nc = _prog_cache[cw]
    res = run_bass_kernel_spmd(nc, in_maps, list(range(NCORES)))
    outs = []
    for cid in range(NCORES):
        o = res.results[cid]["out"].reshape(NWIN * 128, 128)
        outs.append(o[:NPC])
    return np.concatenate(outs, axis=0)[:N_NODES].astype(np.float32)

